# revision 10
# baseline (speedup 1.0000x reference)
"""GATv2 message-passing block on 8 Trainium2 NeuronCores (Bass/Tile).

Graph-parallel, dst-sorted strategy:
  * Host sorts edges by destination node, shards nodes + incoming edges across
    8 cores, and permutes/transposes edge_attr so the device streams it
    feature-major at full DMA rate.
  * Each core builds a node table TBL[n] = [u(32) | xl(96) | xr(96) | pad]
    (u = x@W_edge[:64]+b_edge, xl = x@W_l+b_l, xr = x@W_r+b_r) with 1KB rows,
    plus a core-local XR table (512B rows) for dst-side gathers.
  * Per-edge gathers use InstDMAGatherAnt (dma_gather): int16 indices, so src
    gathers are split into four 32768-row buckets of TBL (edge order inside a
    segment is free -> host packs edges bucket-major); dst gathers index the
    core-local XR table directly.
  * Edge-major compute per 512-edge group: e = relu(u_src + ea@W2),
    eW = e@W_e, m = xl+xr+eW, leaky via max(x, 0.2x) on the vector engine,
    logits = reduce(leaky*att), ex = exp(logits) (softmax max-subtraction
    skipped: softmax is shift-invariant, values are O(1)), msg = ex*xl.
  * Segment reduction by dst via selection-matrix matmuls (Sel[e,n] =
    (dst_local[e]==n), built with iota+is_equal); PSUM accumulates
    [128 nodes x (96 msg + 3 ex)] per segment (<=2048 edges, <=128 nodes,
    cut at node boundaries), scattered to a DRAM accumulator with
    data-driven row targets (each node row written exactly once).
  * Node pass: gat = num/den, x_new = relu(gat@W_n2[:96] + glob2[batch])
    (glob/bias terms folded host-side), per-graph partial sums via a second
    selection matmul; host finishes the tiny global MLP from partials.
"""

import math
import numpy as np
from contextlib import ExitStack
from dataclasses import dataclass

P = 128
BUCKET = 32768         # int16-addressable row bucket for src gathers


@dataclass
class _Cfg:
    N: int = 100000
    E: int = 1600000
    B: int = 64
    n_cores: int = 8
    seg_real: int = 2048        # max real edges per segment
    caps: tuple = (768, 768, 768, 256)   # src bucket slot capacities (128-mults)
    neg_slope: float = 0.2

    @property
    def slots(self):
        return sum(self.caps)   # slot count per segment (multiple of 512)

    @property
    def npc(self):
        return math.ceil(self.N / self.n_cores)

    @property
    def node_slots(self):
        return math.ceil(self.npc / P) * P

    @property
    def nblk(self):
        return self.node_slots // P

    @property
    def tbl_rows(self):
        return math.ceil(self.N / P) * P

    @property
    def nbuck(self):
        return math.ceil(self.tbl_rows / BUCKET)

    @property
    def ch(self):
        return self.slots // P

    @property
    def ngr(self):
        return self.slots // 512


def _prepare(inputs, cfg):
    x = np.ascontiguousarray(np.asarray(inputs["x"], np.float32))
    ea = np.asarray(inputs["edge_attr"], np.float32)
    glob = np.asarray(inputs["glob"], np.float32)
    W_edge = np.asarray(inputs["W_edge"], np.float32)
    b_edge = np.asarray(inputs["b_edge"], np.float32)
    W_l = np.asarray(inputs["W_l"], np.float32)
    b_l = np.asarray(inputs["b_l"], np.float32)
    W_r = np.asarray(inputs["W_r"], np.float32)
    b_r = np.asarray(inputs["b_r"], np.float32)
    W_e = np.asarray(inputs["W_e"], np.float32)
    att = np.asarray(inputs["att"], np.float32)
    bias_gat = np.asarray(inputs["bias_gat"], np.float32)
    W_n2 = np.asarray(inputs["W_n2"], np.float32)
    b_n2 = np.asarray(inputs["b_n2"], np.float32)
    edge_index = np.asarray(inputs["edge_index"]).astype(np.int64)
    batch = np.asarray(inputs["batch"]).astype(np.int64)

    N, NC = cfg.N, cfg.n_cores
    SEG_R, CAPS, SLOTS, CH = cfg.seg_real, cfg.caps, cfg.slots, cfg.ch
    NODE_SLOTS, NPC = cfg.node_slots, cfg.npc

    src = edge_index[0]
    dst = edge_index[1]
    perm = np.argsort(dst, kind="stable")
    src_s = src[perm].astype(np.int64)
    dst_s = dst[perm]

    deg = np.bincount(dst_s, minlength=N)
    cum = np.concatenate([[0], np.cumsum(deg)]).astype(np.int64)

    core_segs = []
    for c in range(NC):
        n0, n1 = c * NPC, min((c + 1) * NPC, N)
        segs = []
        n = n0
        while n < n1:
            m_edge = int(np.searchsorted(cum, cum[n] + SEG_R, side="right")) - 1
            m = min(n + P, n1, m_edge)
            assert m > n, f"node {n} has degree > {SEG_R}"
            segs.append((n, m))
            n = m
        core_segs.append(segs)
    NSEG = max(len(s) for s in core_segs)

    # shared weights
    W_all = np.zeros((65, 256), np.float32)
    W_all[:64, 0:32] = W_edge[:64]
    W_all[64, 0:32] = b_edge
    W_all[:64, 32:128] = W_l
    W_all[64, 32:128] = b_l
    W_all[:64, 128:224] = W_r
    W_all[64, 128:224] = b_r
    W2 = np.ascontiguousarray(W_edge[64:96])
    att_rep = np.tile(att.reshape(1, -1), (P, 1)).astype(np.float32)
    W_n2a = np.ascontiguousarray(W_n2[:96])
    glob2 = (glob @ W_n2[96:128] + b_n2 + bias_gat @ W_n2[:96]).astype(np.float32)

    xT = np.zeros((64, cfg.tbl_rows), np.float32)
    xT[:, :N] = x.T

    # meta int32 cols: src idx (i16 pairs), dst idx (i16 pairs), dloc f32
    # bits per chunk, rowtgt
    IC = SLOTS // 32           # i32 cols per i16 index block
    MC = 2 * IC + CH + 1
    in_maps = []
    # slot bookkeeping for unsharding e: per core/seg, the slot index of each
    # real edge (sorted-order position a+k -> slot)
    slot_maps = []
    cap_off = np.concatenate([[0], np.cumsum(CAPS)]).astype(np.int64)

    for c in range(NC):
        segs = core_segs[c]
        n0, n1 = c * NPC, min((c + 1) * NPC, N)
        meta = np.zeros((NSEG, P, MC), np.int32)
        eaT = np.zeros((NSEG, 32, SLOTS), np.float32)
        pad_queue = list(range(n1 - n0, NODE_SLOTS))
        smaps = []
        for s in range(NSEG):
            src_i16 = np.zeros(SLOTS, np.int16)
            dst_i16 = np.zeros(SLOTS, np.int16)
            dloc = np.full(SLOTS, 999.0, np.float32)
            if s < len(segs):
                n_lo, n_hi = segs[s]
                a, b = int(cum[n_lo]), int(cum[n_hi])
                es, ed = src_s[a:b], dst_s[a:b]
                bucket = (es // BUCKET).astype(np.int64)
                order = np.argsort(bucket, kind="stable")
                cnts = np.bincount(bucket, minlength=len(CAPS))
                assert (cnts <= np.asarray(CAPS)).all(), (
                    f"bucket overflow core{c} seg{s}: {cnts}")
                # slot for each (bucket-sorted) edge
                slots_sorted = np.concatenate([
                    cap_off[bk] + np.arange(cnts[bk]) for bk in range(len(CAPS))
                ]).astype(np.int64) if len(es) else np.zeros(0, np.int64)
                slot_of_edge = np.empty(len(es), np.int64)
                slot_of_edge[order] = slots_sorted
                src_i16[slot_of_edge] = (es % BUCKET).astype(np.int16)
                dst_i16[slot_of_edge] = (ed - n0).astype(np.int16)
                dloc[slot_of_edge] = (ed - n_lo).astype(np.float32)
                eaT[s][:, slot_of_edge] = ea[perm[a:b]].T
                span = n_hi - n_lo
                smaps.append(slot_of_edge)
            else:
                n_lo = n0
                span = 0
                smaps.append(None)
            # idx j -> [j%16, j//16] per bucket region, replicated to 128 parts
            def pack16(v):
                w = v.reshape(-1, 16).T          # [16, SLOTS//16]
                return np.tile(w, (8, 1))        # [128, SLOTS//16]
            meta[s, :, 0:IC] = np.ascontiguousarray(pack16(src_i16)).view(np.int32)
            meta[s, :, IC:2 * IC] = np.ascontiguousarray(pack16(dst_i16)).view(np.int32)
            meta[s, :, 2 * IC:2 * IC + CH] = dloc.reshape(CH, P).T.view(np.int32)
            rt = np.empty(P, np.int32)
            rt[:span] = (n_lo - n0) + np.arange(span)
            for p in range(span, P):
                rt[p] = pad_queue.pop(0) if pad_queue else NODE_SLOTS + p
            meta[s, :, 2 * IC + CH] = rt
        assert not pad_queue
        slot_maps.append(smaps)

        ge2 = np.zeros((NODE_SLOTS, 32), np.float32)
        ge2[: n1 - n0] = glob2[batch[n0:n1]]
        bf = np.full((NODE_SLOTS, 1), 999.0, np.float32)
        bf[: n1 - n0, 0] = batch[n0:n1].astype(np.float32)
        # XR table row targets: phase A2 gathers TBL rows of local nodes
        lrows = np.zeros((NODE_SLOTS + P, 1), np.int32)
        lrows[: n1 - n0, 0] = np.arange(n0, n1, dtype=np.int32)

        in_maps.append({
            "xT": xT, "W_all": W_all, "W2": W2, "W_e": W_e, "W_n2a": W_n2a,
            "att_rep": att_rep, "meta": meta, "eaT": eaT,
            "glob_exp2": ge2, "batchf": bf, "lrows": lrows,
        })
    unshard = {"core_segs": core_segs, "perm": perm, "cum": cum,
               "NSEG": NSEG, "slot_maps": slot_maps}
    return in_maps, unshard


def _build_program(cfg, NSEG, phases="ABCF"):
    import concourse.bass as bass
    import concourse.bacc as bacc
    import concourse.tile as tile
    import concourse.mybir as mybir
    from concourse.masks import make_identity

    f32 = mybir.dt.float32
    i32 = mybir.dt.int32
    i16 = mybir.dt.int16
    SLOTS, CH, NGR = cfg.slots, cfg.ch, cfg.ngr
    CAPS = cfg.caps
    NODE_SLOTS, NBLK, TBL_ROWS = cfg.node_slots, cfg.nblk, cfg.tbl_rows
    GRP = 4
    IC = SLOTS // 32
    MC = 2 * IC + CH + 1
    XR_ROWS = NODE_SLOTS + P
    cap_off = [0]
    for cp in CAPS:
        cap_off.append(cap_off[-1] + cp)

    nc = bacc.Bacc("TRN2", target_bir_lowering=False, debug=False)

    xT_d = nc.dram_tensor("xT", [64, TBL_ROWS], f32, kind="ExternalInput")
    W_all_d = nc.dram_tensor("W_all", [65, 256], f32, kind="ExternalInput")
    W2_d = nc.dram_tensor("W2", [32, 32], f32, kind="ExternalInput")
    W_e_d = nc.dram_tensor("W_e", [32, 96], f32, kind="ExternalInput")
    W_n2a_d = nc.dram_tensor("W_n2a", [96, 32], f32, kind="ExternalInput")
    att_d = nc.dram_tensor("att_rep", [P, 96], f32, kind="ExternalInput")
    meta_d = nc.dram_tensor("meta", [NSEG, P, MC], i32, kind="ExternalInput")
    eaT_d = nc.dram_tensor("eaT", [NSEG, 32, SLOTS], f32, kind="ExternalInput")
    ge2_d = nc.dram_tensor("glob_exp2", [NODE_SLOTS, 32], f32, kind="ExternalInput")
    bf_d = nc.dram_tensor("batchf", [NODE_SLOTS, 1], f32, kind="ExternalInput")
    lrows_d = nc.dram_tensor("lrows", [XR_ROWS, 1], i32, kind="ExternalInput")

    e_outT = nc.dram_tensor("e_outT", [NSEG, 32, SLOTS], f32, kind="ExternalOutput")
    xn_out = nc.dram_tensor("xn_out", [NODE_SLOTS, 32], f32, kind="ExternalOutput")
    gsum_out = nc.dram_tensor("gsum_out", [64, 32], f32, kind="ExternalOutput")

    TBL = nc.dram_tensor("TBL", [TBL_ROWS, 256], f32)
    XRT = nc.dram_tensor("XRT", [XR_ROWS, 128], f32)
    accD = nc.dram_tensor("accD", [NODE_SLOTS + 2 * P, 99], f32)

    with tile.TileContext(nc) as tc, ExitStack() as ctx:
        const = ctx.enter_context(tc.tile_pool(name="const", bufs=1))
        pA = ctx.enter_context(tc.tile_pool(name="pA", bufs=3))
        ps = ctx.enter_context(tc.tile_pool(name="ps", bufs=2, space="PSUM"))
        pB = ctx.enter_context(tc.tile_pool(name="pB", bufs=2))
        pC = ctx.enter_context(tc.tile_pool(name="pC", bufs=2))
        psG = ctx.enter_context(tc.tile_pool(name="psG", bufs=1, space="PSUM"))

        ident = const.tile([P, P], f32)
        make_identity(nc, ident[:])
        iota_i = const.tile([P, P], i32)
        nc.gpsimd.iota(iota_i[:], pattern=[[1, P]], base=0, channel_multiplier=0)
        iota_f = const.tile([P, P], f32)
        nc.vector.tensor_copy(out=iota_f[:], in_=iota_i[:])

        W_all_t = const.tile([65, 256], f32)
        nc.sync.dma_start(out=W_all_t[:], in_=W_all_d.ap())
        W2_t = const.tile([32, 32], f32)
        nc.sync.dma_start(out=W2_t[:], in_=W2_d.ap())
        W_e_t = const.tile([32, 96], f32)
        nc.sync.dma_start(out=W_e_t[:], in_=W_e_d.ap())
        W_n2a_t = const.tile([96, 32], f32)
        nc.sync.dma_start(out=W_n2a_t[:], in_=W_n2a_d.ap())
        att_t = const.tile([P, 96], f32)
        nc.sync.dma_start(out=att_t[:], in_=att_d.ap())

        # ---------------- Phase A: node table ----------------
        for t in range(TBL_ROWS // P if "A" in phases else 0):
            xTa = pA.tile([65, P], f32, tag="xTa")
            nc.sync.dma_start(out=xTa[0:64, :], in_=xT_d.ap()[:, t * P:(t + 1) * P])
            nc.gpsimd.memset(xTa[64:65, :], 1.0)
            tbl_ps = ps.tile([P, 256], f32, tag="ps1")
            nc.tensor.matmul(out=tbl_ps[:], lhsT=xTa[:], rhs=W_all_t[:],
                             start=True, stop=True)
            tbl_sb = pA.tile([P, 256], f32, tag="tbls")
            if t % 2 == 0:
                nc.vector.tensor_copy(out=tbl_sb[:], in_=tbl_ps[:])
            else:
                nc.scalar.copy(out=tbl_sb[:], in_=tbl_ps[:])
            nc.sync.dma_start(out=TBL.ap()[t * P:(t + 1) * P, :], in_=tbl_sb[:])

        # ------------- Phase A2: core-local XR table -------------
        for t in range(XR_ROWS // P if "A" in phases else 0):
            lr_t = pA.tile([P, 1], i32, tag="lr")
            nc.sync.dma_start(out=lr_t[:], in_=lrows_d.ap()[t * P:(t + 1) * P, :])
            xr_t = pA.tile([P, 128], f32, tag="xr")
            nc.gpsimd.indirect_dma_start(
                out=xr_t[:], out_offset=None, in_=TBL.ap(),
                in_offset=bass.IndirectOffsetOnAxis(ap=lr_t[:, 0:1], axis=0),
                element_offset=128)
            nc.sync.dma_start(out=XRT.ap()[t * P:(t + 1) * P, :], in_=xr_t[:])

        # ---------------- Phase B: edges ----------------
        # prime gather tiles so hole slots always hold finite data
        for _ in range(2 if "B" in phases else 0):
            S_p = pB.tile([P, CH, P], f32, tag="S")
            nc.vector.memset(S_p[:], 0.0)
            X_p = pB.tile([P, CH, P], f32, tag="X")
            nc.gpsimd.memset(X_p[:], 0.0)

        for s in range(((1 if "1" in phases else NSEG)) if "B" in phases else 0):
            meta_t = pB.tile([P, MC], i32, tag="meta")
            nc.sync.dma_start(out=meta_t[:], in_=meta_d.ap()[s])
            eaT_t = pB.tile([32, SLOTS], f32, tag="eaT")
            nc.sync.dma_start(out=eaT_t[:], in_=eaT_d.ap()[s])
            src_idx = meta_t[:, 0:IC].bitcast(i16)
            dst_idx = meta_t[:, IC:2 * IC].bitcast(i16)
            dloc_f = meta_t[:, 2 * IC:2 * IC + CH].bitcast(f32)

            S_t = pB.tile([P, CH, P], f32, tag="S")
            for bk in range(len(CAPS) if "d" not in phases else 0):
                lo = bk * BUCKET
                hi = min(lo + BUCKET, TBL_ROWS)
                nc.gpsimd.dma_gather(
                    out_ap=S_t[:, cap_off[bk] // P:cap_off[bk + 1] // P, :],
                    in_ap=TBL.ap()[lo:hi, 0:128],
                    idxs_ap=src_idx[:, cap_off[bk] // 16:cap_off[bk + 1] // 16],
                    num_idxs=CAPS[bk], num_idxs_reg=CAPS[bk],
                    elem_size=128, elem_step=256)
            X_t = pB.tile([P, CH, P], f32, tag="X")
            DG = 768
            if "s" not in phases:
                for o in range(0, SLOTS, DG):
                    w = min(DG, SLOTS - o)
                    nc.gpsimd.dma_gather(
                        out_ap=X_t[:, o // P:(o + w) // P, :],
                        in_ap=XRT.ap(), idxs_ap=dst_idx[:, o // 16:(o + w) // 16],
                        num_idxs=w, num_idxs_reg=w, elem_size=128)

            acc_ps = ps.tile([P, 99], f32, tag="acc")
            for g in range(NGR if "F" in phases else 0):
                e_ps = ps.tile([32, 512], f32, tag="ps1")
                for c in range(GRP):
                    cg = g * GRP + c
                    nc.tensor.matmul(
                        out=e_ps[:, c * P:(c + 1) * P], lhsT=W2_t[:],
                        rhs=eaT_t[:, cg * P:(cg + 1) * P],
                        start=True, stop=False)
                    nc.tensor.matmul(
                        out=e_ps[:, c * P:(c + 1) * P],
                        lhsT=S_t[:, cg, 0:32], rhs=ident[:],
                        start=False, stop=True)
                e_sb = pB.tile([32, 512], f32, tag="e_sb")
                nc.scalar.activation(out=e_sb[:], in_=e_ps[:],
                                     func=mybir.ActivationFunctionType.Relu)
                nc.sync.dma_start(out=e_outT.ap()[s, :, g * 512:(g + 1) * 512],
                                  in_=e_sb[:])
                m_ps = ps.tile([P, GRP, 96], f32, tag="ps2")
                for c in range(GRP):
                    cg = g * GRP + c
                    nc.tensor.matmul(
                        out=m_ps[:, c, :], lhsT=e_sb[:, c * P:(c + 1) * P],
                        rhs=W_e_t[:], start=True, stop=False)
                    nc.tensor.matmul(
                        out=m_ps[:, c, :], lhsT=ident[:],
                        rhs=X_t[:, cg, 0:96], start=False, stop=True)
                m_sb = pB.tile([P, GRP, 96], f32, tag="m_sb")
                nc.vector.tensor_tensor(
                    out=m_sb[:], in0=m_ps[:],
                    in1=S_t[:, g * GRP:(g + 1) * GRP, 32:128],
                    op=mybir.AluOpType.add)
                lk = pB.tile([P, GRP, 96], f32, tag="lk")
                nc.vector.scalar_tensor_tensor(
                    out=lk[:], in0=m_sb[:], scalar=cfg.neg_slope, in1=m_sb[:],
                    op0=mybir.AluOpType.mult, op1=mybir.AluOpType.max)
                at = pB.tile([P, GRP, 96], f32, tag="at")
                nc.vector.tensor_tensor(
                    out=at[:], in0=lk[:],
                    in1=att_t[:].unsqueeze(1).to_broadcast([P, GRP, 96]),
                    op=mybir.AluOpType.mult)
                R_t = pB.tile([P, GRP, 99], f32, tag="R")
                lg = pB.tile([P, GRP, 3], f32, tag="lg")
                nc.vector.tensor_reduce(
                    out=lg[:],
                    in_=at[:].rearrange("p c (h k) -> p c h k", h=3),
                    axis=mybir.AxisListType.X, op=mybir.AluOpType.add)
                nc.scalar.activation(out=R_t[:, :, 96:99], in_=lg[:],
                                     func=mybir.ActivationFunctionType.Exp)
                nc.vector.tensor_tensor(
                    out=R_t[:, :, 0:96].rearrange("p c (h k) -> p c h k", h=3),
                    in0=S_t[:, g * GRP:(g + 1) * GRP, 32:128]
                        .rearrange("p c (h k) -> p c h k", h=3),
                    in1=R_t[:, :, 96:99].unsqueeze(-1).to_broadcast([P, GRP, 3, 32]),
                    op=mybir.AluOpType.mult)
                sel = pB.tile([P, GRP, P], f32, tag="sel")
                nc.vector.tensor_tensor(
                    out=sel[:],
                    in0=dloc_f[:, g * GRP:(g + 1) * GRP].unsqueeze(-1)
                        .to_broadcast([P, GRP, P]),
                    in1=iota_f[:].unsqueeze(1).to_broadcast([P, GRP, P]),
                    op=mybir.AluOpType.is_equal)
                for c in range(GRP):
                    nc.tensor.matmul(
                        out=acc_ps[:], lhsT=sel[:, c, :], rhs=R_t[:, c, :],
                        start=(g == 0 and c == 0),
                        stop=(g == NGR - 1 and c == GRP - 1))
            if "F" not in phases:
                continue
            acc_sb = pB.tile([P, 99], f32, tag="acc_sb")
            nc.vector.tensor_copy(out=acc_sb[:], in_=acc_ps[:])
            nc.gpsimd.indirect_dma_start(
                out=accD.ap(),
                out_offset=bass.IndirectOffsetOnAxis(
                    ap=meta_t[:, 2 * IC + CH:2 * IC + CH + 1], axis=0),
                in_=acc_sb[:], in_offset=None)

        # ---------------- Phase C: nodes ----------------
        gsum_ps = psG.tile([64, 32], f32)
        for b in range(NBLK if "C" in phases else 0):
            acc_t = pC.tile([P, 99], f32, tag="acc_t")
            nc.sync.dma_start(out=acc_t[:], in_=accD.ap()[b * P:(b + 1) * P, :])
            ge_t = pC.tile([P, 32], f32, tag="ge")
            nc.sync.dma_start(out=ge_t[:], in_=ge2_d.ap()[b * P:(b + 1) * P, :])
            bf_t = pC.tile([P, 1], f32, tag="bf")
            nc.sync.dma_start(out=bf_t[:], in_=bf_d.ap()[b * P:(b + 1) * P, :])
            den_t = pC.tile([P, 3], f32, tag="den")
            nc.vector.tensor_scalar_max(den_t[:], acc_t[:, 96:99], 1e-30)
            rec_t = pC.tile([P, 3], f32, tag="rec")
            nc.vector.reciprocal(out=rec_t[:], in_=den_t[:])
            gat_t = pC.tile([P, 96], f32, tag="gat")
            nc.vector.tensor_tensor(
                out=gat_t[:].rearrange("p (h k) -> p h k", h=3),
                in0=acc_t[:, 0:96].rearrange("p (h k) -> p h k", h=3),
                in1=rec_t[:].unsqueeze(-1).to_broadcast([P, 3, 32]),
                op=mybir.AluOpType.mult)
            gT_ps = ps.tile([96, P], f32, tag="ps1")
            nc.tensor.matmul(out=gT_ps[:], lhsT=gat_t[:], rhs=ident[:],
                             start=True, stop=True)
            gT_sb = pC.tile([96, P], f32, tag="gTs")
            nc.scalar.copy(out=gT_sb[:], in_=gT_ps[:])
            xn_ps = ps.tile([P, 32], f32, tag="ps2")
            nc.tensor.matmul(out=xn_ps[:], lhsT=gT_sb[:], rhs=W_n2a_t[:],
                             start=True, stop=False)
            nc.tensor.matmul(out=xn_ps[:], lhsT=ident[:], rhs=ge_t[:],
                             start=False, stop=True)
            xn_sb = pC.tile([P, 32], f32, tag="xns")
            nc.scalar.activation(out=xn_sb[:], in_=xn_ps[:],
                                 func=mybir.ActivationFunctionType.Relu)
            nc.sync.dma_start(out=xn_out.ap()[b * P:(b + 1) * P, :], in_=xn_sb[:])
            selg = pC.tile([P, 64], f32, tag="selg")
            nc.vector.tensor_tensor(
                out=selg[:], in0=bf_t[:, 0:1].to_broadcast([P, 64]),
                in1=iota_f[:, 0:64], op=mybir.AluOpType.is_equal)
            nc.tensor.matmul(out=gsum_ps[:], lhsT=selg[:], rhs=xn_sb[:],
                             start=(b == 0), stop=(b == NBLK - 1))
        if "C" in phases:
            gsum_sb = pC.tile([64, 32], f32, tag="gsum_sb")
            nc.vector.tensor_copy(out=gsum_sb[:], in_=gsum_ps[:])
            nc.sync.dma_start(out=gsum_out.ap(), in_=gsum_sb[:])

    nc.compile()
    return nc


def _postprocess(inputs, cfg, unshard, results):
    glob = np.asarray(inputs["glob"], np.float32)
    W_g = np.asarray(inputs["W_g"], np.float32)
    b_g = np.asarray(inputs["b_g"], np.float32)
    batch = np.asarray(inputs["batch"]).astype(np.int64)
    perm, cum = unshard["perm"], unshard["cum"]

    NPC = cfg.npc
    xn = np.concatenate([
        results[c]["xn_out"][: min((c + 1) * NPC, cfg.N) - c * NPC]
        for c in range(cfg.n_cores)
    ])
    e_full = np.empty((cfg.E, 32), np.float32)
    for c in range(cfg.n_cores):
        eo = results[c]["e_outT"]
        for s, (n_lo, n_hi) in enumerate(unshard["core_segs"][c]):
            a, b = int(cum[n_lo]), int(cum[n_hi])
            slots = unshard["slot_maps"][c][s]
            e_full[perm[a:b]] = eo[s][:, slots].T
    gsum = np.sum([results[c]["gsum_out"] for c in range(cfg.n_cores)], axis=0)[: cfg.B]
    cnt = np.bincount(batch, minlength=cfg.B).astype(np.float32)
    mean = gsum / np.maximum(cnt, 1.0)[:, None]
    u_new = np.maximum(np.concatenate([glob, mean], axis=1) @ W_g + b_g, 0.0)
    return xn, e_full, u_new.astype(np.float32)


def _run(inputs, cfg, trace=False):
    from concourse.bass_utils import run_bass_kernel_spmd

    in_maps, unshard = _prepare(inputs, cfg)
    nc = _build_program(cfg, unshard["NSEG"])
    res = run_bass_kernel_spmd(nc, in_maps, list(range(cfg.n_cores)), trace=trace)
    return _postprocess(inputs, cfg, unshard, res.results), res


def kernel(**inputs):
    (xn, e_full, u_new), _ = _run(inputs, _Cfg(), trace=False)
    return xn, e_full, u_new
